# revision 4
# baseline (speedup 1.0000x reference)
"""ANFIS Trainium2 kernel (8 NeuronCores, Bass/Tile).

Math (reference):
  mfs[b,i,j] = exp(-(x[b,i]-centers[i,j])^2 / (2*widths[i,j]^2))   [1024,8,4]
  w[b,r]     = prod_i mfs[b,i,idx_i(r)]    r in [0, 4^8=65536), i0 slowest
  w        <- w / sum_r w
  out[b,n]   = sum_r w[b,r] * ([x[b],1] . rule_params[r,:,n])      [1024,16]

Key structure: w = wA (x) wB with wA over dims 0..2 (64 vals), wB over dims
3..7 (1024 vals); r = rA*1024 + rB.  The denominator factorizes:
sum_r w = prod_i (sum_j mfs[b,i,j]).

Sharding: rA split across 8 cores (8 local rA each = contiguous 8192-rule
row blocks of rule_params).  Per core:
  out_c[b,n] = sum_{rA local} wA[b,rA]/denom[b] *
               sum_i xb[b,i] (sum_rB wB[b,rB] rp[rA*1024+rB, i*16+n])
followed by a ReduceScatter(add) over cores; each core emits its 128-row
batch shard, host concatenates.

The inner sum_rB is a matmul with contraction over rB; it needs wB^T
([rB, b] layout, rB on partitions).  wB is built in [b, rB] layout with
strided free-axis outer products, then transposed via PE matmuls against
per-(b-tile, i7) diagonal matrices diag(mfs7) -- this fuses the last
Kronecker factor (dim 7) into the transpose:
  out[q, b] = sum_b' w3456[b', q] * diag(mfs7)[b', b] = w3456[b,q]*mfs7[b]
rB is enumerated as rB' = i7*256 + q (q = dims 3..6); rule_params rows are
permuted on the host to match.
"""

import sys

sys.path.insert(0, "/opt/trn_rl_repo")

import numpy as np

import concourse.bass as bass
import concourse.bacc as bacc
import concourse.tile as tile
import concourse.mybir as mybir
from concourse.ap import AP
from concourse.bass_utils import run_bass_kernel_spmd

F32 = mybir.dt.float32
MULT = mybir.AluOpType.mult
ADD = mybir.AluOpType.add
SUB = mybir.AluOpType.subtract
EXP = mybir.ActivationFunctionType.Exp

N_CORES = 8
B = 1024
BT = 8          # batch tiles of 128
D = 8           # input dims
M = 4           # membership fns per dim
NO = 16         # outputs
C = (D + 1) * NO            # 144
NRA = 64        # 4^3 (dims 0..2)
RA_LOC = NRA // N_CORES     # 8 local rA per core
NRB = 1024      # 4^5 (dims 3..7)
KT = 8          # rB partition tiles of 128
# rA groups per psum bank (N<=512 fp32): {0,1,2},{3,4,5},{6,7}
GROUPS = [(0, 3), (3, 3), (6, 2)]


def _v(t, off, dims):
    """Custom free-dim view of a [128, F] SBUF tile AP.

    t: AP covering the full tile ([128, F]); off: element offset within the
    partition row; dims: list of (step, count) free dims, outer..inner.
    """
    part = list(t.ap[0])
    return AP(
        tensor=t.tensor,
        offset=t.offset + off,
        ap=[part] + [[s, n] for (s, n) in dims],
    )


def build_nc():
    nc = bacc.Bacc("TRN2", target_bir_lowering=False, debug=False,
                   num_devices=N_CORES)

    x_all_d = nc.declare_dram_parameter("x_all", [128, BT * D], F32, isOutput=False)
    cb_d = nc.declare_dram_parameter("cb", [128, D * M], F32, isOutput=False)
    wt_d = nc.declare_dram_parameter("wt", [128, D * M], F32, isOutput=False)
    cA_d = nc.declare_dram_parameter("cA", [128, RA_LOC * 3], F32, isOutput=False)
    wtA_d = nc.declare_dram_parameter("wtA", [128, RA_LOC * 3], F32, isOutput=False)
    eye_d = nc.declare_dram_parameter("eye", [128, 128], F32, isOutput=False)
    rp_d = nc.declare_dram_parameter("rp", [128, KT * RA_LOC * C], F32, isOutput=False)
    out_d = nc.declare_dram_parameter("out", [B // N_CORES, NO], F32, isOutput=True)

    with tile.TileContext(nc) as tc:
        with (
            tc.tile_pool(name="const", bufs=1) as cpool,
            tc.tile_pool(name="rp", bufs=1) as rppool,
            tc.tile_pool(name="wbt", bufs=1) as wbtpool,
            tc.tile_pool(name="work", bufs=3) as work,
            tc.tile_pool(name="dtile", bufs=3) as dpool,
            tc.tile_pool(name="acc", bufs=2) as accpool,
            tc.tile_pool(name="psT", bufs=2, space="PSUM") as psT,
            tc.tile_pool(name="ps0", bufs=2, space="PSUM") as ps0p,
            tc.tile_pool(name="ps1", bufs=2, space="PSUM") as ps1p,
            tc.tile_pool(name="ps2", bufs=2, space="PSUM") as ps2p,
            tc.tile_pool(name="dram", bufs=1, space="DRAM") as dram,
        ):
            # ---- load constants / inputs ----
            xa = cpool.tile([128, BT * D], F32, tag="xa")
            cb = cpool.tile([128, D * M], F32, tag="cb")
            wt = cpool.tile([128, D * M], F32, tag="wt")
            cA = cpool.tile([128, RA_LOC * 3], F32, tag="cA")
            wtA = cpool.tile([128, RA_LOC * 3], F32, tag="wtA")
            eye = cpool.tile([128, 128], F32, tag="eye")
            rp = rppool.tile([128, KT * RA_LOC * C], F32, tag="rp")

            nc.sync.dma_start(xa[:], x_all_d[:])
            nc.sync.dma_start(cb[:], cb_d[:])
            nc.sync.dma_start(wt[:], wt_d[:])
            nc.sync.dma_start(cA[:], cA_d[:])
            nc.sync.dma_start(wtA[:], wtA_d[:])
            nc.sync.dma_start(eye[:], eye_d[:])
            SLAB = RA_LOC * C  # 1152 elems per kt slab
            for kt in range(KT):
                nc.sync.dma_start(rp[:, kt * SLAB:(kt + 1) * SLAB],
                                  rp_d[:, kt * SLAB:(kt + 1) * SLAB])

            # ---- full membership values mfs [128, (bt, i, j)] ----
            # nw = -1/(2 w^2) for all (i,j)
            t32a = work.tile([128, D * M], F32, tag="t32")
            t32b = work.tile([128, D * M], F32, tag="t32")
            nw = cpool.tile([128, D * M], F32, tag="nw")
            nc.vector.tensor_tensor(t32a[:], wt[:], wt[:], op=MULT)
            nc.vector.tensor_scalar_mul(t32b[:], t32a[:], -2.0)
            nc.vector.reciprocal(nw[:], t32b[:])

            MF = BT * D * M  # 256
            dif = work.tile([128, MF], F32, tag="dif")
            # dif[p, bt*32+i*4+j] = x[bt,i] - cb[i*4+j]
            nc.vector.tensor_tensor(
                dif[:],
                _v(xa[:], 0, [(D, BT), (1, D), (0, M)]),
                _v(cb[:], 0, [(0, BT), (1, D * M)]),
                op=SUB,
            )
            d2 = work.tile([128, MF], F32, tag="d2")
            nc.vector.tensor_tensor(d2[:], dif[:], dif[:], op=MULT)
            d2s = work.tile([128, MF], F32, tag="d2s")
            nc.vector.tensor_tensor(
                d2s[:], d2[:], _v(nw[:], 0, [(0, BT), (1, D * M)]), op=MULT)
            mfs = cpool.tile([128, MF], F32, tag="mfs")
            nc.scalar.activation(mfs[:], d2s[:], EXP)

            # ---- local wA [128, (bt, r)] from per-core selected centers ----
            t24a = work.tile([128, RA_LOC * 3], F32, tag="t24")
            t24b = work.tile([128, RA_LOC * 3], F32, tag="t24")
            nwA = cpool.tile([128, RA_LOC * 3], F32, tag="nwA")
            nc.vector.tensor_tensor(t24a[:], wtA[:], wtA[:], op=MULT)
            nc.vector.tensor_scalar_mul(t24b[:], t24a[:], -2.0)
            nc.vector.reciprocal(nwA[:], t24b[:])

            NA = BT * RA_LOC * 3  # 192
            dA = work.tile([128, NA], F32, tag="dA")
            nc.vector.tensor_tensor(
                dA[:],
                _v(xa[:], 0, [(D, BT), (0, RA_LOC), (1, 3)]),
                _v(cA[:], 0, [(0, BT), (3, RA_LOC), (1, 3)]),
                op=SUB,
            )
            dA2 = work.tile([128, NA], F32, tag="dA2")
            nc.vector.tensor_tensor(dA2[:], dA[:], dA[:], op=MULT)
            dA2s = work.tile([128, NA], F32, tag="dA2s")
            nc.vector.tensor_tensor(
                dA2s[:], dA2[:],
                _v(nwA[:], 0, [(0, BT), (3, RA_LOC), (1, 3)]), op=MULT)
            eA = work.tile([128, BT * RA_LOC], F32, tag="eA")
            nc.vector.reduce_sum(
                eA[:], _v(dA2s[:], 0, [(3 * RA_LOC, BT), (3, RA_LOC), (1, 3)]),
                axis=mybir.AxisListType.X)
            wA = cpool.tile([128, BT * RA_LOC], F32, tag="wA")
            nc.scalar.activation(wA[:], eA[:], EXP)

            # ---- denominator: denom[b] = prod_i sum_j mfs ----
            s = work.tile([128, BT * D], F32, tag="s")
            nc.vector.reduce_sum(
                s[:], _v(mfs[:], 0, [(M, BT * D), (1, M)]),
                axis=mybir.AxisListType.X)
            p1 = work.tile([128, BT * 4], F32, tag="p1")
            nc.vector.tensor_tensor(
                p1[:], _v(s[:], 0, [(D, BT), (1, 4)]),
                _v(s[:], 4, [(D, BT), (1, 4)]), op=MULT)
            p2 = work.tile([128, BT * 2], F32, tag="p2")
            nc.vector.tensor_tensor(
                p2[:], _v(p1[:], 0, [(4, BT), (1, 2)]),
                _v(p1[:], 2, [(4, BT), (1, 2)]), op=MULT)
            p3 = work.tile([128, BT], F32, tag="p3")
            nc.vector.tensor_tensor(
                p3[:], _v(p2[:], 0, [(2, BT)]), _v(p2[:], 1, [(2, BT)]),
                op=MULT)
            invd = cpool.tile([128, BT], F32, tag="invd")
            nc.vector.reciprocal(invd[:], p3[:])

            # wAn = wA * invd (per b-tile column of invd)
            wAn = cpool.tile([128, BT * RA_LOC], F32, tag="wAn")
            for bt in range(BT):
                nc.vector.tensor_scalar_mul(
                    wAn[:, bt * RA_LOC:(bt + 1) * RA_LOC],
                    wA[:, bt * RA_LOC:(bt + 1) * RA_LOC],
                    invd[:, bt:bt + 1])

            # ---- wB factors over dims 3..6 in [b, (bt, q)] layout ----
            # mfs col offsets: dim k lives at i=k -> offset k*M within a bt block
            w34 = work.tile([128, BT * 16], F32, tag="w34")
            nc.vector.tensor_tensor(
                w34[:],
                _v(mfs[:], 3 * M, [(D * M, BT), (1, M), (0, M)]),
                _v(mfs[:], 4 * M, [(D * M, BT), (0, M), (1, M)]),
                op=MULT)
            w56 = work.tile([128, BT * 16], F32, tag="w56")
            nc.vector.tensor_tensor(
                w56[:],
                _v(mfs[:], 5 * M, [(D * M, BT), (1, M), (0, M)]),
                _v(mfs[:], 6 * M, [(D * M, BT), (0, M), (1, M)]),
                op=MULT)
            w3456 = cpool.tile([128, BT * 256], F32, tag="w3456")
            nc.vector.tensor_tensor(
                w3456[:],
                _v(w34[:], 0, [(16, BT), (1, 16), (0, 16)]),
                _v(w56[:], 0, [(16, BT), (0, 16), (1, 16)]),
                op=MULT)

            # ---- wB^T via PE transpose against diag(mfs7): wbt[kt][q, b] ----
            wbt = [wbtpool.tile([128, B], F32, tag=f"wbt{kt}", name=f"wbt{kt}")
                   for kt in range(KT)]
            for bt in range(BT):
                for j in range(M):
                    dj = dpool.tile([128, 128], F32, tag="dj")
                    nc.vector.tensor_scalar_mul(
                        dj[:], eye[:], mfs[:, bt * D * M + 7 * M + j:
                                           bt * D * M + 7 * M + j + 1])
                    for qh in range(2):
                        pT = psT.tile([128, 128], F32, tag="pT")
                        nc.tensor.matmul(
                            pT[:],
                            w3456[:, bt * 256 + qh * 128: bt * 256 + (qh + 1) * 128],
                            dj[:], start=True, stop=True)
                        kt = 2 * j + qh
                        nc.scalar.copy(wbt[kt][:, bt * 128:(bt + 1) * 128], pT[:])

            # ---- main matmuls + evac ----
            partial = dram.tile([B, NO], F32)
            for bt in range(BT):
                ps = [ps0p.tile([128, GROUPS[0][1] * C], F32, tag="ps0", name="ps0"),
                      ps1p.tile([128, GROUPS[1][1] * C], F32, tag="ps1", name="ps1"),
                      ps2p.tile([128, GROUPS[2][1] * C], F32, tag="ps2", name="ps2")]
                for kt in range(KT):
                    lhsT = wbt[kt][:, bt * 128:(bt + 1) * 128]
                    for g, (r0, nr) in enumerate(GROUPS):
                        nc.tensor.matmul(
                            ps[g][:], lhsT,
                            _v(rp[:], (kt * RA_LOC + r0) * C, [(C, nr), (1, C)]),
                            start=(kt == 0), stop=(kt == KT - 1))
                acc = accpool.tile([128, C], F32, tag="acc")
                for r in range(RA_LOC):
                    g, idx = r // 3, r % 3
                    seg = ps[g][:, idx * C:(idx + 1) * C]
                    sc = wAn[:, bt * RA_LOC + r: bt * RA_LOC + r + 1]
                    if r == 0:
                        nc.vector.tensor_scalar_mul(acc[:], seg, sc)
                    else:
                        nc.vector.scalar_tensor_tensor(
                            acc[:], seg, sc, acc[:], op0=MULT, op1=ADD)
                # xb contraction: out[b,n] = sum_i x[b,i]*acc[:,i*16+n] + acc[:,128+n]
                t16 = accpool.tile([128, NO], F32, tag="t16")
                nc.vector.tensor_scalar_mul(
                    t16[:], acc[:, 0:NO], xa[:, bt * D: bt * D + 1])
                for i in range(1, D):
                    nc.vector.scalar_tensor_tensor(
                        t16[:], acc[:, i * NO:(i + 1) * NO],
                        xa[:, bt * D + i: bt * D + i + 1], t16[:],
                        op0=MULT, op1=ADD)
                ob = accpool.tile([128, NO], F32, tag="ob")
                nc.vector.tensor_tensor(
                    ob[:], t16[:], acc[:, D * NO:(D + 1) * NO], op=ADD)
                nc.sync.dma_start(partial[bt * 128:(bt + 1) * 128, :], ob[:])

            # ---- reduce-scatter over cores; each core keeps its b shard ----
            rs_out = dram.tile([B // N_CORES, NO], F32)
            nc.gpsimd.collective_compute(
                "ReduceScatter", ADD,
                replica_groups=[list(range(N_CORES))],
                ins=[partial.opt()], outs=[rs_out.opt()])
            nc.sync.dma_start(out_d[:], rs_out[:])

    nc.compile()
    return nc


_NC_CACHE = None


def _get_nc():
    global _NC_CACHE
    if _NC_CACHE is None:
        _NC_CACHE = build_nc()
    return _NC_CACHE


def _prep_in_maps(x, centers, widths, rule_params):
    x = np.asarray(x, np.float32)
    centers = np.asarray(centers, np.float32)
    widths = np.asarray(widths, np.float32)
    rule_params = np.asarray(rule_params, np.float32)

    # x_all[p, bt*8+i] = x[bt*128+p, i]
    x_all = np.ascontiguousarray(
        x.reshape(BT, 128, D).transpose(1, 0, 2).reshape(128, BT * D))
    cb = np.ascontiguousarray(
        np.broadcast_to(centers.reshape(1, D * M), (128, D * M)))
    wt = np.ascontiguousarray(
        np.broadcast_to(widths.reshape(1, D * M), (128, D * M)))
    eye = np.eye(128, dtype=np.float32)

    # rule_params rows r = rA*1024 + q*4 + j  ->  per core [p, kt, rA, c]
    # with row order rB' = j*256 + q, kt = rB' tile of 128.
    rp4 = rule_params.reshape(NRA, 256, M, C).transpose(0, 2, 1, 3)
    rp4 = rp4.reshape(NRA, NRB, C)  # rows rB' = j*256+q

    in_maps = []
    for c in range(N_CORES):
        ra0 = c * RA_LOC
        # selected centers/widths for local rA triples (dims 0..2)
        idx = np.empty((RA_LOC, 3), np.int64)
        for r in range(RA_LOC):
            ra = ra0 + r
            idx[r] = [(ra >> 4) & 3, (ra >> 2) & 3, ra & 3]
        k = np.arange(3)
        cA = centers[k[None, :], idx]        # [RA_LOC, 3]
        wtA = widths[k[None, :], idx]
        cA = np.ascontiguousarray(
            np.broadcast_to(cA.reshape(1, RA_LOC * 3), (128, RA_LOC * 3)))
        wtA = np.ascontiguousarray(
            np.broadcast_to(wtA.reshape(1, RA_LOC * 3), (128, RA_LOC * 3)))

        rp_c = rp4[ra0:ra0 + RA_LOC]                     # [8, 1024, 144]
        rp_c = rp_c.reshape(RA_LOC, KT, 128, C).transpose(2, 1, 0, 3)
        rp_c = np.ascontiguousarray(rp_c.reshape(128, KT * RA_LOC * C))

        in_maps.append({
            "x_all": x_all, "cb": cb, "wt": wt,
            "cA": cA, "wtA": wtA, "eye": eye, "rp": rp_c,
        })
    return in_maps


def kernel(x, centers, widths, rule_params, _trace=False):
    nc = _get_nc()
    in_maps = _prep_in_maps(x, centers, widths, rule_params)
    res = run_bass_kernel_spmd(nc, in_maps, core_ids=list(range(N_CORES)),
                               trace=_trace)
    out = np.concatenate([res.results[c]["out"] for c in range(N_CORES)],
                         axis=0)
    if _trace:
        kernel._last_exec_time_ns = res.exec_time_ns
        kernel._last_results = res
    return out


# revision 6
# speedup vs baseline: 2.4115x; 2.4115x over previous
"""ANFIS Trainium2 kernel (8 NeuronCores, Bass/Tile).

Math (reference):
  mfs[b,i,j] = exp(-(x[b,i]-centers[i,j])^2 / (2*widths[i,j]^2))   [1024,8,4]
  w[b,r]     = prod_i mfs[b,i,idx_i(r)]    r in [0, 4^8=65536), i0 slowest
  w        <- w / sum_r w
  out[b,n]   = sum_r w[b,r] * ([x[b],1] . rule_params[r,:,n])      [1024,16]

Key structure: w = wA (x) wB with wA over dims 0..2 (64 vals), wB over dims
3..7 (1024 vals); r = rA*1024 + rB.  The denominator factorizes:
sum_r w = prod_i (sum_j mfs[b,i,j]).

Sharding: rA split across 8 cores (8 local rA each = contiguous 8192-rule
row blocks of rule_params).  Per core:
  out_c[b,n] = sum_{rA local} wA[b,rA]/denom[b] *
               sum_i xb[b,i] (sum_rB wB[b,rB] rp[rA*1024+rB, i*16+n])
followed by a ReduceScatter(add) over cores; each core emits its 128-row
batch shard, host concatenates.

The inner sum_rB is a matmul with contraction over rB; it needs wB^T
([rB, b] layout, rB on partitions).  wB is built in [b, rB] layout with
strided free-axis outer products, then transposed via PE matmuls against
per-(b-tile, i7) diagonal matrices diag(mfs7) -- this fuses the last
Kronecker factor (dim 7) into the transpose:
  out[q, b] = sum_b' w3456[b', q] * diag(mfs7)[b', b] = w3456[b,q]*mfs7[b]
rB is enumerated as rB' = i7*256 + q (q = dims 3..6); rule_params rows are
permuted on the host to match.
"""

import sys

sys.path.insert(0, "/opt/trn_rl_repo")

import numpy as np

import concourse.bass as bass
import concourse.bacc as bacc
import concourse.tile as tile
import concourse.mybir as mybir
from concourse.ap import AP
from concourse.bass_utils import run_bass_kernel_spmd

F32 = mybir.dt.float32
BF16 = mybir.dt.bfloat16
MULT = mybir.AluOpType.mult
ADD = mybir.AluOpType.add
SUB = mybir.AluOpType.subtract
EXP = mybir.ActivationFunctionType.Exp

N_CORES = 8
B = 1024
BT = 8          # batch tiles of 128
D = 8           # input dims
M = 4           # membership fns per dim
NO = 16         # outputs
C = (D + 1) * NO            # 144
NRA = 64        # 4^3 (dims 0..2)
RA_LOC = NRA // N_CORES     # 8 local rA per core
NRB = 1024      # 4^5 (dims 3..7)
KT = 8          # rB partition tiles of 128
# rA groups per psum bank (N<=512 fp32): {0,1,2},{3,4,5},{6,7}
GROUPS = [(0, 3), (3, 3), (6, 2)]


def _v(t, off, dims):
    """Custom free-dim view of a [128, F] SBUF tile AP.

    t: AP covering the full tile ([128, F]); off: element offset within the
    partition row; dims: list of (step, count) free dims, outer..inner.
    """
    part = list(t.ap[0])
    return AP(
        tensor=t.tensor,
        offset=t.offset + off,
        ap=[part] + [[s, n] for (s, n) in dims],
    )


def build_nc():
    nc = bacc.Bacc("TRN2", target_bir_lowering=False, debug=False,
                   num_devices=N_CORES)

    x_all_d = nc.declare_dram_parameter("x_all", [128, BT * D], F32, isOutput=False)
    cb_d = nc.declare_dram_parameter("cb", [128, D * M], F32, isOutput=False)
    wt_d = nc.declare_dram_parameter("wt", [128, D * M], F32, isOutput=False)
    cA_d = nc.declare_dram_parameter("cA", [128, RA_LOC * 3], F32, isOutput=False)
    wtA_d = nc.declare_dram_parameter("wtA", [128, RA_LOC * 3], F32, isOutput=False)
    eye_d = nc.declare_dram_parameter("eye", [128, 128], BF16, isOutput=False)
    rp_d = nc.declare_dram_parameter("rp", [128, KT * RA_LOC * C], BF16, isOutput=False)
    out_d = nc.declare_dram_parameter("out", [B // N_CORES, NO], F32, isOutput=True)

    with tile.TileContext(nc) as tc:
        with (
            tc.tile_pool(name="const", bufs=1) as cpool,
            tc.tile_pool(name="rp", bufs=1) as rppool,
            tc.tile_pool(name="wbt", bufs=1) as wbtpool,
            tc.tile_pool(name="work", bufs=3) as work,
            tc.tile_pool(name="dtile", bufs=3) as dpool,
            tc.tile_pool(name="acc", bufs=2) as accpool,
            tc.tile_pool(name="psT", bufs=2, space="PSUM") as psT,
            tc.tile_pool(name="ps0", bufs=2, space="PSUM") as ps0p,
            tc.tile_pool(name="ps1", bufs=2, space="PSUM") as ps1p,
            tc.tile_pool(name="ps2", bufs=2, space="PSUM") as ps2p,
            tc.tile_pool(name="dram", bufs=1, space="DRAM") as dram,
        ):
            # ---- load constants / inputs ----
            xa = cpool.tile([128, BT * D], F32, tag="xa")
            cb = cpool.tile([128, D * M], F32, tag="cb")
            wt = cpool.tile([128, D * M], F32, tag="wt")
            cA = cpool.tile([128, RA_LOC * 3], F32, tag="cA")
            wtA = cpool.tile([128, RA_LOC * 3], F32, tag="wtA")
            eye = cpool.tile([128, 128], BF16, tag="eye")
            rp = rppool.tile([128, KT * RA_LOC * C], BF16, tag="rp")

            nc.sync.dma_start(xa[:], x_all_d[:])
            nc.sync.dma_start(cb[:], cb_d[:])
            nc.sync.dma_start(wt[:], wt_d[:])
            nc.sync.dma_start(cA[:], cA_d[:])
            nc.sync.dma_start(wtA[:], wtA_d[:])
            nc.sync.dma_start(eye[:], eye_d[:])
            SLAB = RA_LOC * C  # 1152 elems per kt slab
            for kt in range(KT):
                nc.sync.dma_start(rp[:, kt * SLAB:(kt + 1) * SLAB],
                                  rp_d[:, kt * SLAB:(kt + 1) * SLAB])

            # ---- full membership values mfs [128, (bt, i, j)] ----
            # nw = -1/(2 w^2) for all (i,j)
            t32a = work.tile([128, D * M], F32, tag="t32")
            t32b = work.tile([128, D * M], F32, tag="t32")
            nw = cpool.tile([128, D * M], F32, tag="nw")
            nc.vector.tensor_tensor(t32a[:], wt[:], wt[:], op=MULT)
            nc.vector.tensor_scalar_mul(t32b[:], t32a[:], -2.0)
            nc.vector.reciprocal(nw[:], t32b[:])

            MF = BT * D * M  # 256
            dif = work.tile([128, MF], F32, tag="dif")
            # dif[p, bt*32+i*4+j] = x[bt,i] - cb[i*4+j]
            nc.vector.tensor_tensor(
                dif[:],
                _v(xa[:], 0, [(D, BT), (1, D), (0, M)]),
                _v(cb[:], 0, [(0, BT), (1, D * M)]),
                op=SUB,
            )
            d2 = work.tile([128, MF], F32, tag="d2")
            nc.vector.tensor_tensor(d2[:], dif[:], dif[:], op=MULT)
            d2s = work.tile([128, MF], F32, tag="d2s")
            nc.vector.tensor_tensor(
                d2s[:], d2[:], _v(nw[:], 0, [(0, BT), (1, D * M)]), op=MULT)
            mfs = cpool.tile([128, MF], F32, tag="mfs")
            nc.scalar.activation(mfs[:], d2s[:], EXP)
            mfsb = cpool.tile([128, MF], BF16, tag="mfsb")
            nc.vector.tensor_copy(mfsb[:], mfs[:])

            # ---- local wA [128, (bt, r)] from per-core selected centers ----
            t24a = work.tile([128, RA_LOC * 3], F32, tag="t24")
            t24b = work.tile([128, RA_LOC * 3], F32, tag="t24")
            nwA = cpool.tile([128, RA_LOC * 3], F32, tag="nwA")
            nc.vector.tensor_tensor(t24a[:], wtA[:], wtA[:], op=MULT)
            nc.vector.tensor_scalar_mul(t24b[:], t24a[:], -2.0)
            nc.vector.reciprocal(nwA[:], t24b[:])

            NA = BT * RA_LOC * 3  # 192
            dA = work.tile([128, NA], F32, tag="dA")
            nc.vector.tensor_tensor(
                dA[:],
                _v(xa[:], 0, [(D, BT), (0, RA_LOC), (1, 3)]),
                _v(cA[:], 0, [(0, BT), (3, RA_LOC), (1, 3)]),
                op=SUB,
            )
            dA2 = work.tile([128, NA], F32, tag="dA2")
            nc.vector.tensor_tensor(dA2[:], dA[:], dA[:], op=MULT)
            dA2s = work.tile([128, NA], F32, tag="dA2s")
            nc.vector.tensor_tensor(
                dA2s[:], dA2[:],
                _v(nwA[:], 0, [(0, BT), (3, RA_LOC), (1, 3)]), op=MULT)
            eA = work.tile([128, BT * RA_LOC], F32, tag="eA")
            nc.vector.reduce_sum(
                eA[:], _v(dA2s[:], 0, [(3 * RA_LOC, BT), (3, RA_LOC), (1, 3)]),
                axis=mybir.AxisListType.X)
            wA = cpool.tile([128, BT * RA_LOC], F32, tag="wA")
            nc.scalar.activation(wA[:], eA[:], EXP)

            # ---- denominator: denom[b] = prod_i sum_j mfs ----
            s = work.tile([128, BT * D], F32, tag="s")
            nc.vector.reduce_sum(
                s[:], _v(mfs[:], 0, [(M, BT * D), (1, M)]),
                axis=mybir.AxisListType.X)
            p1 = work.tile([128, BT * 4], F32, tag="p1")
            nc.vector.tensor_tensor(
                p1[:], _v(s[:], 0, [(D, BT), (1, 4)]),
                _v(s[:], 4, [(D, BT), (1, 4)]), op=MULT)
            p2 = work.tile([128, BT * 2], F32, tag="p2")
            nc.vector.tensor_tensor(
                p2[:], _v(p1[:], 0, [(4, BT), (1, 2)]),
                _v(p1[:], 2, [(4, BT), (1, 2)]), op=MULT)
            p3 = work.tile([128, BT], F32, tag="p3")
            nc.vector.tensor_tensor(
                p3[:], _v(p2[:], 0, [(2, BT)]), _v(p2[:], 1, [(2, BT)]),
                op=MULT)
            invd = cpool.tile([128, BT], F32, tag="invd")
            nc.vector.reciprocal(invd[:], p3[:])

            # wAn = wA * invd (per b-tile column of invd)
            wAn = cpool.tile([128, BT * RA_LOC], F32, tag="wAn")
            for bt in range(BT):
                nc.vector.tensor_scalar_mul(
                    wAn[:, bt * RA_LOC:(bt + 1) * RA_LOC],
                    wA[:, bt * RA_LOC:(bt + 1) * RA_LOC],
                    invd[:, bt:bt + 1])

            # ---- wB factors over dims 3..6 in [b, (bt, q)] layout ----
            # mfs col offsets: dim k lives at i=k -> offset k*M within a bt block
            w34 = work.tile([128, BT * 16], BF16, tag="w34")
            nc.vector.tensor_tensor(
                w34[:],
                _v(mfsb[:], 3 * M, [(D * M, BT), (1, M), (0, M)]),
                _v(mfsb[:], 4 * M, [(D * M, BT), (0, M), (1, M)]),
                op=MULT)
            w56 = work.tile([128, BT * 16], BF16, tag="w56")
            nc.vector.tensor_tensor(
                w56[:],
                _v(mfsb[:], 5 * M, [(D * M, BT), (1, M), (0, M)]),
                _v(mfsb[:], 6 * M, [(D * M, BT), (0, M), (1, M)]),
                op=MULT)
            w3456 = cpool.tile([128, BT * 256], BF16, tag="w3456")
            nc.vector.tensor_tensor(
                w3456[:],
                _v(w34[:], 0, [(16, BT), (1, 16), (0, 16)]),
                _v(w56[:], 0, [(16, BT), (0, 16), (1, 16)]),
                op=MULT)

            # ---- wB^T via PE transpose against diag(mfs7): wbt[kt][q, b] ----
            wbt = [wbtpool.tile([128, B], BF16, tag=f"wbt{kt}", name=f"wbt{kt}")
                   for kt in range(KT)]
            for bt in range(BT):
                for j in range(M):
                    dj = dpool.tile([128, 128], BF16, tag="dj")
                    nc.vector.tensor_scalar_mul(
                        dj[:], eye[:], mfs[:, bt * D * M + 7 * M + j:
                                           bt * D * M + 7 * M + j + 1])
                    for qh in range(2):
                        pT = psT.tile([128, 128], F32, tag="pT")
                        nc.tensor.matmul(
                            pT[:],
                            w3456[:, bt * 256 + qh * 128: bt * 256 + (qh + 1) * 128],
                            dj[:], start=True, stop=True)
                        kt = 2 * j + qh
                        nc.scalar.copy(wbt[kt][:, bt * 128:(bt + 1) * 128], pT[:])

            # ---- main matmuls + evac ----
            partial = dram.tile([B, NO], F32)
            for bt in range(BT):
                ps = [ps0p.tile([128, GROUPS[0][1] * C], F32, tag="ps0", name="ps0"),
                      ps1p.tile([128, GROUPS[1][1] * C], F32, tag="ps1", name="ps1"),
                      ps2p.tile([128, GROUPS[2][1] * C], F32, tag="ps2", name="ps2")]
                for kt in range(KT):
                    lhsT = wbt[kt][:, bt * 128:(bt + 1) * 128]
                    for g, (r0, nr) in enumerate(GROUPS):
                        nc.tensor.matmul(
                            ps[g][:], lhsT,
                            _v(rp[:], (kt * RA_LOC + r0) * C, [(C, nr), (1, C)]),
                            start=(kt == 0), stop=(kt == KT - 1))
                acc = accpool.tile([128, C], F32, tag="acc")
                for r in range(RA_LOC):
                    g, idx = r // 3, r % 3
                    seg = ps[g][:, idx * C:(idx + 1) * C]
                    sc = wAn[:, bt * RA_LOC + r: bt * RA_LOC + r + 1]
                    if r == 0:
                        nc.vector.tensor_scalar_mul(acc[:], seg, sc)
                    else:
                        nc.vector.scalar_tensor_tensor(
                            acc[:], seg, sc, acc[:], op0=MULT, op1=ADD)
                # xb contraction: out[b,n] = sum_i x[b,i]*acc[:,i*16+n] + acc[:,128+n]
                t16 = accpool.tile([128, NO], F32, tag="t16")
                nc.vector.tensor_scalar_mul(
                    t16[:], acc[:, 0:NO], xa[:, bt * D: bt * D + 1])
                for i in range(1, D):
                    nc.vector.scalar_tensor_tensor(
                        t16[:], acc[:, i * NO:(i + 1) * NO],
                        xa[:, bt * D + i: bt * D + i + 1], t16[:],
                        op0=MULT, op1=ADD)
                ob = accpool.tile([128, NO], F32, tag="ob")
                nc.vector.tensor_tensor(
                    ob[:], t16[:], acc[:, D * NO:(D + 1) * NO], op=ADD)
                nc.sync.dma_start(partial[bt * 128:(bt + 1) * 128, :], ob[:])

            # ---- reduce-scatter over cores; each core keeps its b shard ----
            rs_out = dram.tile([B // N_CORES, NO], F32)
            nc.gpsimd.collective_compute(
                "ReduceScatter", ADD,
                replica_groups=[list(range(N_CORES))],
                ins=[partial.opt()], outs=[rs_out.opt()])
            nc.sync.dma_start(out_d[:], rs_out[:])

    nc.compile()
    return nc


_NC_CACHE = None


def _get_nc():
    global _NC_CACHE
    if _NC_CACHE is None:
        _NC_CACHE = build_nc()
    return _NC_CACHE


def _prep_in_maps(x, centers, widths, rule_params):
    x = np.asarray(x, np.float32)
    centers = np.asarray(centers, np.float32)
    widths = np.asarray(widths, np.float32)
    rule_params = np.asarray(rule_params, np.float32)

    # x_all[p, bt*8+i] = x[bt*128+p, i]
    x_all = np.ascontiguousarray(
        x.reshape(BT, 128, D).transpose(1, 0, 2).reshape(128, BT * D))
    cb = np.ascontiguousarray(
        np.broadcast_to(centers.reshape(1, D * M), (128, D * M)))
    wt = np.ascontiguousarray(
        np.broadcast_to(widths.reshape(1, D * M), (128, D * M)))
    import ml_dtypes
    eye = np.eye(128, dtype=ml_dtypes.bfloat16)

    # rule_params rows r = rA*1024 + q*4 + j  ->  per core [p, kt, rA, c]
    # with row order rB' = j*256 + q, kt = rB' tile of 128.
    rp4 = rule_params.reshape(NRA, 256, M, C).transpose(0, 2, 1, 3)
    rp4 = rp4.reshape(NRA, NRB, C)  # rows rB' = j*256+q

    in_maps = []
    for c in range(N_CORES):
        ra0 = c * RA_LOC
        # selected centers/widths for local rA triples (dims 0..2)
        idx = np.empty((RA_LOC, 3), np.int64)
        for r in range(RA_LOC):
            ra = ra0 + r
            idx[r] = [(ra >> 4) & 3, (ra >> 2) & 3, ra & 3]
        k = np.arange(3)
        cA = centers[k[None, :], idx]        # [RA_LOC, 3]
        wtA = widths[k[None, :], idx]
        cA = np.ascontiguousarray(
            np.broadcast_to(cA.reshape(1, RA_LOC * 3), (128, RA_LOC * 3)))
        wtA = np.ascontiguousarray(
            np.broadcast_to(wtA.reshape(1, RA_LOC * 3), (128, RA_LOC * 3)))

        rp_c = rp4[ra0:ra0 + RA_LOC]                     # [8, 1024, 144]
        rp_c = rp_c.reshape(RA_LOC, KT, 128, C).transpose(2, 1, 0, 3)
        rp_c = np.ascontiguousarray(rp_c.reshape(128, KT * RA_LOC * C)).astype(ml_dtypes.bfloat16)

        in_maps.append({
            "x_all": x_all, "cb": cb, "wt": wt,
            "cA": cA, "wtA": wtA, "eye": eye, "rp": rp_c,
        })
    return in_maps


def kernel(x, centers, widths, rule_params, _trace=False):
    nc = _get_nc()
    in_maps = _prep_in_maps(x, centers, widths, rule_params)
    res = run_bass_kernel_spmd(nc, in_maps, core_ids=list(range(N_CORES)),
                               trace=_trace)
    out = np.concatenate([res.results[c]["out"] for c in range(N_CORES)],
                         axis=0)
    if _trace:
        kernel._last_exec_time_ns = res.exec_time_ns
        kernel._last_results = res
    return out
